# revision 2
# baseline (speedup 1.0000x reference)
"""Trainium2 Bass kernel for CrossChannelInterp.

Reference computation (per batch b, timestep t; D=128 channels):
    y      = x[..., :D]                       observed values
    w      = x[..., D:2D]                     log-intensities
    y_obs  = x[..., 2D:3D]                    transient targets
    intensity = exp(w)
    sm     = softmax(w, axis=channels)
    mean   = mean_t y                         (per-batch, per-channel)
    smooth = (sm * (y - mean)) @ kernel + mean
    y_trans = y_obs - smooth
    out    = concat([smooth, intensity, y_trans], -1)

Sharding: pure data parallel over the batch dim across 8 cores
(16 batches per core); the 128x128 kernel matrix is replicated.

Per-core kernel structure (B_CORE=16 batches, T=1024 in 8 tiles of 128):
  - softmax is computed un-normalized (e = exp(w), no max subtraction —
    inputs are ~N(0,1) so exp can't overflow); the 1/sum factor is applied
    *after* the channel matmul as a per-partition scale:
        smooth = (e*(y-mean) @ K) * r + mean,   r = 1/sum_i e
  - mean is computed on the TensorEngine as ones(1/T).T @ y accumulated
    over the 8 t-tiles -> a (1,128) row, then broadcast to a (128,128)
    tile with a rank-1 matmul against a ones row.
  - a = e*(y-mean) is transposed per 128x128 tile on the TensorEngine
    (PE transpose via identity) since the matmul contracts over channels.
"""

import threading

import numpy as np

B, T, D = 128, 1024, 128
N_CORES = 8
B_CORE = B // N_CORES  # 16 batches per core
NT = T // 128  # 8 t-tiles per batch
C3 = 3 * D  # 384

_lock = threading.Lock()
_cache = {}


def _build_nc():
    from contextlib import ExitStack

    import concourse.bacc as bacc
    import concourse.tile as tile
    from concourse import mybir

    fp32 = mybir.dt.float32

    nc = bacc.Bacc(
        "TRN2",
        target_bir_lowering=False,
        debug=False,
        num_devices=N_CORES,
    )

    x = nc.dram_tensor("x", [B_CORE, T, C3], fp32, kind="ExternalInput")
    kmat = nc.dram_tensor("kernel", [D, D], fp32, kind="ExternalInput")
    out = nc.dram_tensor("out", [B_CORE, T, C3], fp32, kind="ExternalOutput")

    ident_dram = nc.inline_tensor(np.eye(D, dtype=np.float32), name="ident")

    # DRAM views: t = n*128 + p, partition = p, free = (n, c)
    xr = x.ap().rearrange("b (n p) c -> b p n c", p=128)
    outr = out.ap().rearrange("b (n p) c -> b p n c", p=128)

    Exp = mybir.ActivationFunctionType.Exp
    sub = mybir.AluOpType.subtract
    mult = mybir.AluOpType.mult

    with tile.TileContext(nc) as tc, ExitStack() as ctx:
        singles = ctx.enter_context(tc.tile_pool(name="singles", bufs=1))
        inpool = ctx.enter_context(tc.tile_pool(name="inpool", bufs=3))
        outpool = ctx.enter_context(tc.tile_pool(name="outpool", bufs=3))
        tmppool = ctx.enter_context(tc.tile_pool(name="tmppool", bufs=2))
        statpool = ctx.enter_context(tc.tile_pool(name="statpool", bufs=2))
        atsbpool = ctx.enter_context(tc.tile_pool(name="atsb", bufs=3))
        ps_mean = ctx.enter_context(tc.tile_pool(name="ps_mean", bufs=2, space="PSUM"))
        ps_bc = ctx.enter_context(tc.tile_pool(name="ps_bc", bufs=2, space="PSUM"))
        ps_at = ctx.enter_context(tc.tile_pool(name="ps_at", bufs=2, space="PSUM"))
        ps_mm = ctx.enter_context(tc.tile_pool(name="ps_mm", bufs=2, space="PSUM"))

        # constants
        k_sb = singles.tile([D, D], fp32)
        nc.sync.dma_start(out=k_sb[:], in_=kmat.ap())
        ident_sb = singles.tile([D, D], fp32)
        nc.sync.dma_start(out=ident_sb[:], in_=ident_dram.ap())
        ones_col = singles.tile([128, 1], fp32)  # 1/T for the mean matmul
        nc.vector.memset(ones_col[:], 1.0 / T)
        ones_row = singles.tile([1, 128], fp32)
        nc.vector.memset(ones_row[:], 1.0)

        for b in range(B_CORE):
            in_tile = inpool.tile([128, NT, C3], fp32)
            nc.sync.dma_start(out=in_tile[:], in_=xr[b])

            y_v = in_tile[:, :, 0:D]
            w_v = in_tile[:, :, D : 2 * D]
            yobs_v = in_tile[:, :, 2 * D : 3 * D]

            # ---- mean over T: ones(1/T).T @ y accumulated over t-tiles ----
            mean_ps = ps_mean.tile([1, D], fp32)
            for n in range(NT):
                nc.tensor.matmul(
                    mean_ps[:],
                    lhsT=ones_col[:],
                    rhs=in_tile[:, n, 0:D],
                    start=(n == 0),
                    stop=(n == NT - 1),
                )
            meanrow = statpool.tile([1, D], fp32)
            nc.scalar.copy(meanrow[:], mean_ps[:])

            # broadcast the mean row to all 128 partitions (rank-1 matmul)
            bc_ps = ps_bc.tile([128, D], fp32)
            nc.tensor.matmul(bc_ps[:], lhsT=ones_row[:], rhs=meanrow[:])
            bc_sb = statpool.tile([128, D], fp32)
            nc.scalar.copy(bc_sb[:], bc_ps[:])
            bc_bcast = (
                bc_sb[:]
                .rearrange("p (o c) -> p o c", o=1)
                .to_broadcast([128, NT, D])
            )

            out_tile = outpool.tile([128, NT, C3], fp32)
            e_v = out_tile[:, :, D : 2 * D]  # intensity block doubles as e

            # e = exp(w), written straight into the output tile
            nc.scalar.activation(e_v, w_v, Exp)

            # s = sum_i e ; r = 1/s
            s_t = statpool.tile([128, NT], fp32)
            nc.vector.reduce_sum(s_t[:], e_v, axis=mybir.AxisListType.X)
            r_t = statpool.tile([128, NT], fp32)
            nc.vector.reciprocal(r_t[:], s_t[:])

            # a = e * (y - mean)
            t1 = tmppool.tile([128, NT, D], fp32)
            nc.vector.tensor_tensor(t1[:], y_v, bc_bcast, op=sub)
            a_t = tmppool.tile([128, NT, D], fp32)
            nc.vector.tensor_tensor(a_t[:], e_v, t1[:], op=mult)

            # per t-tile: transpose a, matmul with kernel, scale by r
            for n in range(NT):
                at_ps = ps_at.tile([128, D], fp32)
                nc.tensor.transpose(at_ps[:], a_t[:, n, :], ident_sb[:])
                at_sb = atsbpool.tile([128, D], fp32)
                nc.scalar.copy(at_sb[:], at_ps[:])
                mm_ps = ps_mm.tile([128, D], fp32)
                nc.tensor.matmul(mm_ps[:], lhsT=at_sb[:], rhs=k_sb[:])
                # smooth (pre-mean) = mm * r  (per-partition scale on ACT)
                nc.scalar.activation(
                    out_tile[:, n, 0:D],
                    mm_ps[:],
                    mybir.ActivationFunctionType.Copy,
                    scale=r_t[:, n : n + 1],
                )

            # smooth += mean (broadcast add over all 8 tiles at once)
            nc.vector.tensor_tensor(
                out_tile[:, :, 0:D], out_tile[:, :, 0:D], bc_bcast, op=mybir.AluOpType.add
            )
            # y_trans = y_obs - smooth
            nc.vector.tensor_tensor(
                out_tile[:, :, 2 * D : 3 * D], yobs_v, out_tile[:, :, 0:D], op=sub
            )

            nc.sync.dma_start(out=outr[b], in_=out_tile[:])

    nc.finalize()
    return nc


def _get_nc():
    with _lock:
        if "nc" not in _cache:
            _cache["nc"] = _build_nc()
        return _cache["nc"]


def kernel(x: np.ndarray, kernel: np.ndarray) -> np.ndarray:
    from concourse.bass_utils import run_bass_kernel_spmd

    x = np.ascontiguousarray(x, dtype=np.float32)
    kmat = np.ascontiguousarray(kernel, dtype=np.float32)
    nc = _get_nc()

    in_maps = [
        {"x": x[c * B_CORE : (c + 1) * B_CORE], "kernel": kmat}
        for c in range(N_CORES)
    ]
    res = run_bass_kernel_spmd(nc, in_maps, core_ids=list(range(N_CORES)))
    return np.concatenate([r["out"] for r in res.results], axis=0)


if __name__ == "__main__":
    rng = np.random.default_rng(0)
    x = rng.standard_normal((B, T, C3), dtype=np.float32)
    km = np.eye(D, dtype=np.float32) + 0.01 * rng.standard_normal((D, D), dtype=np.float32)
    out = kernel(x, km)
    print(out.shape, out.dtype)


# revision 6
# speedup vs baseline: 1.1611x; 1.1611x over previous
"""Trainium2 Bass kernel for CrossChannelInterp.

Reference computation (per batch b, timestep t; D=128 channels):
    y      = x[..., :D]                       observed values
    w      = x[..., D:2D]                     log-intensities
    y_obs  = x[..., 2D:3D]                    transient targets
    intensity = exp(w)
    sm     = softmax(w, axis=channels)
    mean   = mean_t y                         (per-batch, per-channel)
    smooth = (sm * (y - mean)) @ kernel + mean
    y_trans = y_obs - smooth
    out    = concat([smooth, intensity, y_trans], -1)

Sharding: pure data parallel over the batch dim across 8 cores
(16 batches per core); the 128x128 kernel matrix is replicated.

Per-core kernel structure (B_CORE=16 batches, T=1024 in 8 tiles of 128):
  - softmax is computed un-normalized (e = exp(w), no max subtraction —
    inputs are ~N(0,1) so exp can't overflow); the 1/sum factor is applied
    *after* the channel matmul as a per-partition scale:
        smooth = (e*(y-mean) @ K) * r + mean,   r = 1/sum_i e
  - mean is computed on the TensorEngine as ones(1/T).T @ y accumulated
    over the 8 t-tiles -> a (1,128) row, then broadcast to a (128,128)
    tile with a rank-1 matmul against a ones row.
  - a = e*(y-mean) is transposed per 128x128 tile on the TensorEngine
    (PE transpose via identity) since the matmul contracts over channels.
"""

import threading

import numpy as np

B, T, D = 128, 1024, 128
N_CORES = 8
B_CORE = B // N_CORES  # 16 batches per core
NT = T // 128  # 8 t-tiles per batch
C3 = 3 * D  # 384

_lock = threading.Lock()
_cache = {}


def _build_nc():
    from contextlib import ExitStack

    import concourse.bacc as bacc
    import concourse.tile as tile
    from concourse import mybir

    fp32 = mybir.dt.float32

    nc = bacc.Bacc(
        "TRN2",
        target_bir_lowering=False,
        debug=False,
        num_devices=N_CORES,
    )

    x = nc.dram_tensor("x", [B_CORE, T, C3], fp32, kind="ExternalInput")
    kmat = nc.dram_tensor("kernel", [D, D], fp32, kind="ExternalInput")
    out = nc.dram_tensor("out", [B_CORE, T, C3], fp32, kind="ExternalOutput")

    import ml_dtypes

    ident_dram = nc.inline_tensor(np.eye(D, dtype=ml_dtypes.bfloat16), name="ident")
    # every row = 1/T: one matmul ones_scaled.T @ s1 broadcasts the mean
    ones_dram = nc.inline_tensor(
        np.full((128, 128), 1.0 / T, dtype=np.float32), name="ones_scaled"
    )

    # DRAM views: t = n*128 + p, partition = p, free = (n, c)
    xr = x.ap().rearrange("b (n p) c -> b p n c", p=128)
    outr = out.ap().rearrange("b (n p) c -> b p n c", p=128)

    Exp = mybir.ActivationFunctionType.Exp
    sub = mybir.AluOpType.subtract
    mult = mybir.AluOpType.mult

    with tile.TileContext(nc) as tc, ExitStack() as ctx:
        singles = ctx.enter_context(tc.tile_pool(name="singles", bufs=1))
        inpool = ctx.enter_context(tc.tile_pool(name="inpool", bufs=3))
        outpool = ctx.enter_context(tc.tile_pool(name="outpool", bufs=3))
        tmppool = ctx.enter_context(tc.tile_pool(name="tmppool", bufs=2))
        statpool = ctx.enter_context(tc.tile_pool(name="statpool", bufs=2))
        atsbpool = ctx.enter_context(tc.tile_pool(name="atsb", bufs=3))
        ps_bc = ctx.enter_context(tc.tile_pool(name="ps_bc", bufs=2, space="PSUM"))
        ps_at = ctx.enter_context(tc.tile_pool(name="ps_at", bufs=3, space="PSUM"))
        ps_mm = ctx.enter_context(tc.tile_pool(name="ps_mm", bufs=3, space="PSUM"))

        # constants
        bf16 = mybir.dt.bfloat16
        k_sb = singles.tile([D, D], fp32)
        nc.sync.dma_start(out=k_sb[:], in_=kmat.ap())
        k_bf = singles.tile([D, D], bf16)
        nc.vector.tensor_copy(k_bf[:], k_sb[:])
        ident_sb = singles.tile([D, D], bf16)
        nc.sync.dma_start(out=ident_sb[:], in_=ident_dram.ap())
        ones_sb = singles.tile([128, 128], fp32)
        nc.sync.dma_start(out=ones_sb[:], in_=ones_dram.ap())

        for b in range(B_CORE):
            in_tile = inpool.tile([128, NT, C3], fp32)
            nc.sync.dma_start(out=in_tile[:], in_=xr[b])

            y_v = in_tile[:, :, 0:D]
            w_v = in_tile[:, :, D : 2 * D]
            yobs_v = in_tile[:, :, 2 * D : 3 * D]

            # ---- mean over T, broadcast to all partitions ----
            # s1[p, c] = sum_n y[p, n, c]  (DVE reduce over the tile index)
            s1 = tmppool.tile([128, D], fp32)
            nc.vector.reduce_sum(
                s1[:], y_v.rearrange("p n c -> p c n"), axis=mybir.AxisListType.X
            )
            # bc[t, c] = sum_p s1[p, c] / T  — every row = mean
            bc_ps = ps_bc.tile([128, D], fp32)
            nc.tensor.matmul(bc_ps[:], lhsT=ones_sb[:], rhs=s1[:])
            bc_sb = statpool.tile([128, D], fp32)
            nc.scalar.copy(bc_sb[:], bc_ps[:])
            bc_bcast = (
                bc_sb[:]
                .rearrange("p (o c) -> p o c", o=1)
                .to_broadcast([128, NT, D])
            )

            out_tile = outpool.tile([128, NT, C3], fp32)
            e_v = out_tile[:, :, D : 2 * D]  # intensity block doubles as e

            # e = exp(w) straight into the output tile; accum_out gives the
            # softmax denominators for free
            s_t = statpool.tile([128, NT], fp32)
            for n in range(NT):
                nc.scalar.activation(
                    out_tile[:, n, D : 2 * D],
                    in_tile[:, n, D : 2 * D],
                    Exp,
                    accum_out=s_t[:, n : n + 1],
                )
            r_t = statpool.tile([128, NT], fp32)
            nc.vector.reciprocal(r_t[:], s_t[:])

            # a = e * (y - mean), cast to bf16 for the PE
            t1 = tmppool.tile([128, NT, D], fp32)
            nc.vector.tensor_tensor(t1[:], y_v, bc_bcast, op=sub)
            a_t = tmppool.tile([128, NT, D], bf16)
            nc.vector.tensor_tensor(a_t[:], e_v, t1[:], op=mult)

            # per t-tile: transpose a, matmul with kernel, scale by r, add mean
            for n in range(NT):
                at_ps = ps_at.tile([128, D], bf16)
                nc.tensor.transpose(at_ps[:], a_t[:, n, :], ident_sb[:])
                at_sb = atsbpool.tile([128, D], bf16)
                nc.scalar.copy(at_sb[:], at_ps[:])
                mm_ps = ps_mm.tile([128, D], fp32)
                nc.tensor.matmul(mm_ps[:], lhsT=at_sb[:], rhs=k_bf[:])
                # smooth = mm * r + mean   (fused on DVE, reads PSUM)
                nc.vector.scalar_tensor_tensor(
                    out_tile[:, n, 0:D],
                    mm_ps[:],
                    r_t[:, n : n + 1],
                    bc_sb[:],
                    op0=mult,
                    op1=mybir.AluOpType.add,
                )

            # y_trans = y_obs - smooth
            nc.vector.tensor_tensor(
                out_tile[:, :, 2 * D : 3 * D], yobs_v, out_tile[:, :, 0:D], op=sub
            )

            nc.sync.dma_start(out=outr[b], in_=out_tile[:])

    nc.finalize()
    return nc


def _get_nc():
    with _lock:
        if "nc" not in _cache:
            _cache["nc"] = _build_nc()
        return _cache["nc"]


def kernel(x: np.ndarray, kernel: np.ndarray) -> np.ndarray:
    from concourse.bass_utils import run_bass_kernel_spmd

    x = np.ascontiguousarray(x, dtype=np.float32)
    kmat = np.ascontiguousarray(kernel, dtype=np.float32)
    nc = _get_nc()

    in_maps = [
        {"x": x[c * B_CORE : (c + 1) * B_CORE], "kernel": kmat}
        for c in range(N_CORES)
    ]
    res = run_bass_kernel_spmd(nc, in_maps, core_ids=list(range(N_CORES)))
    return np.concatenate([r["out"] for r in res.results], axis=0)


if __name__ == "__main__":
    rng = np.random.default_rng(0)
    x = rng.standard_normal((B, T, C3), dtype=np.float32)
    km = np.eye(D, dtype=np.float32) + 0.01 * rng.standard_normal((D, D), dtype=np.float32)
    out = kernel(x, km)
    print(out.shape, out.dtype)
